# revision 8
# baseline (speedup 1.0000x reference)
"""Multi-head attention (RoPE + u-bias + bool mask) Trainium2 Bass kernel.

Data-parallel over batch: one batch element per NeuronCore (8 cores).

Design notes (vs the original baseline):
  - Few large DMAs; weights/mask/x loaded exactly once. (The baseline\'s
    751 small DMAs serialized on the shared HWDGE at ~625ns each.)
  - bf16 value path (xv/wv/wo/mask/em/vaug/ctxu), fp32r q/k path.
  - Row-tiled score matmuls (two K=64 heads packed at partition 0/64).
  - ACT does exp only (plus q/k bias evac in the otherwise-idle proj
    phase); all other PSUM evacuation on DVE.
  - Softmax denominator: ones-column appended to the ctx matmul
    stationary; DVE reciprocal straight from PSUM row 64; broadcast
    via a DRAM round-trip.
  - One PSUM pool of four uniform [128,1024] two-bank tags shared by all
    phases: no pool-close engine drains between phases, attention is
    software-pipelined (ctx trails scores by 4 tiles), and the out
    projection starts on the score banks while the last softmax
    epilogue drains.

Shapes: q/k/v (8,1024,1024) f32, mask (8,1024,1024) bool,
W* (1024,1024) f32, b* (1024,) f32, u_bias (16,64) f32 -> out (8,1024,1024) f32.
"""

import sys

if "/opt/trn_rl_repo" not in sys.path:
    sys.path.insert(0, "/opt/trn_rl_repo")

from contextlib import ExitStack

import ml_dtypes
import numpy as np

import concourse.bass as bass
from concourse import bacc
import concourse.tile as tile
from concourse import mybir
from concourse.bass_utils import run_bass_kernel_spmd

B, S, D, H, Dh = 8, 1024, 1024, 16, 64
P = 128
NT = D // P  # 8 tiles along d
ST = S // P  # 8 tiles along s/t
HF = S // 2  # 512 = one PSUM bank of fp32
FP = mybir.dt.float32
BF = mybir.dt.bfloat16
FPR = mybir.dt.float32r
ROPE_BASE = 10000.0
AF = mybir.ActivationFunctionType
ALU = mybir.AluOpType

N_CORES = 8

# DVE writing a 64-partition op to a shifted output quadrant (ctx evac for
# the odd head). Falls back to an ACT partition-shift copy when False.
SHIFT_DVE = True


def build_nc():
    nc = bacc.Bacc("TRN2", target_bir_lowering=False, debug=False)

    # DRAM I/O
    xqT = nc.dram_tensor("xqT", [D, S], FPR, kind="ExternalInput").ap()
    xkT = nc.dram_tensor("xkT", [D, S], FPR, kind="ExternalInput").ap()
    xvT = nc.dram_tensor("xvT", [D, S], BF, kind="ExternalInput").ap()
    wqT = nc.dram_tensor("wqT", [D, D], FPR, kind="ExternalInput").ap()
    wkT = nc.dram_tensor("wkT", [D, D], FPR, kind="ExternalInput").ap()
    wvT = nc.dram_tensor("wvT", [D, D], BF, kind="ExternalInput").ap()
    woT = nc.dram_tensor("woT", [D, D], BF, kind="ExternalInput").ap()
    maskT = nc.dram_tensor("maskT", [S, S], BF, kind="ExternalInput").ap()
    costab = nc.dram_tensor("costab", [P, S], FP, kind="ExternalInput").ap()
    sintab = nc.dram_tensor("sintab", [P, S], FP, kind="ExternalInput").ap()
    # smalls[:, 0:8]=ucols, 8:16=bqcols, 16:24=bkcols
    smalls = nc.dram_tensor("smalls", [P, 24], FP, kind="ExternalInput").ap()
    pswap = nc.dram_tensor("pswap", [P, P], FPR, kind="ExternalInput").ap()
    # rows_bf[0, 0:1024]=bv, 1024:2048=bo, 2048:2176=ones (bf16)
    rows_bf = nc.dram_tensor("rows_bf", [1, 2 * D + P], BF, kind="ExternalInput").ap()
    out = nc.dram_tensor("out", [S, D], FP, kind="ExternalOutput").ap()
    rec_d = nc.dram_tensor("rec_scratch", [H, S], FP).ap()

    with tile.TileContext(nc) as tc, ExitStack() as ctx:
        persist = ctx.enter_context(tc.tile_pool(name="persist", bufs=1))
        ps = ctx.enter_context(tc.tile_pool(name="ps", bufs=1, space="PSUM"))
        # early pool: mask/wo/rb loaded at t=0 into space disjoint from the
        # projection-phase pools, so attention and out-proj never stall on WAR
        pE = ctx.enter_context(tc.tile_pool(name="early", bufs=1))
        # x pool: per-k input tiles; xq loads overlap the V phase
        pX = ctx.enter_context(tc.tile_pool(name="xpool", bufs=1))

        # ---- persistent constants / state ----
        qb_sb = persist.tile([P, NT * S], FPR)  # rope(Q)^T + u, tile j at j*S
        kb_sb = persist.tile([P, NT * S], FPR)  # rope(K)^T
        # V augmented with ones column: [p, st, h, 0:64]=V, [.,.,.,64]=1  (bf16)
        vaug = persist.tile([P, ST * H * (Dh + 1)], BF)
        vaug_v = vaug[:].rearrange("p (st h c) -> p st h c", st=ST, h=H)
        nc.gpsimd.memset(vaug_v[:, :, :, Dh : Dh + 1], 1.0)
        ctxu = persist.tile([P, NT * S], BF)  # normalized ctx^T (bf16)

        # ======== phase V: V projection into vaug ========
        with tc.tile_pool(name="poolV", bufs=1) as pV:
            # first DMAs: k=0 blocks of xv/wv so the first matmuls start early
            xvs, wvs = [], []
            for k in range(NT):
                xvk = pV.tile([P, S], BF, tag=f"xv{k}", name=f"xv{k}")
                wvk = pV.tile([P, D], BF, tag=f"wv{k}", name=f"wv{k}")
                if k == 0:
                    # lead chunks first so the k=0 matmuls start ~2us earlier
                    nc.sync.dma_start(xvk[:, 0:HF], xvT[0:P, 0:HF])
                    nc.sync.dma_start(wvk[:, 0:HF], wvT[0:P, 0:HF])
                    nc.sync.dma_start(xvk[:, HF:S], xvT[0:P, HF:S])
                    nc.sync.dma_start(wvk[:, HF:D], wvT[0:P, HF:D])
                else:
                    nc.sync.dma_start(xvk[:], xvT[k * P : (k + 1) * P, :])
                    nc.sync.dma_start(wvk[:], wvT[k * P : (k + 1) * P, :])
                xvs.append(xvk)
                wvs.append(wvk)

            # constants / rows
            cos_sb = persist.tile([P, S], FP)
            nc.sync.dma_start(cos_sb[:], costab[:])
            sin_sb = persist.tile([P, S], FP)
            nc.sync.dma_start(sin_sb[:], sintab[:])
            smalls_sb = persist.tile([P, 24], FP)
            nc.sync.dma_start(smalls_sb[:], smalls[:])
            ucols_sb = smalls_sb[:, 0:8]
            bqcols_sb = smalls_sb[:, 8:16]
            bkcols_sb = smalls_sb[:, 16:24]
            pswap_sb = persist.tile([P, P], FPR)
            nc.sync.dma_start(pswap_sb[:], pswap[:])
            rows_sb = persist.tile([1, 2 * D + P], BF)
            nc.sync.dma_start(rows_sb[:], rows_bf[:])
            bvrow_sb = rows_sb[:, 0:D]
            borow_sb = rows_sb[:, D : 2 * D]
            ones_row = rows_sb[:, 2 * D : 2 * D + P]

            # early loads (disjoint space): xq, mask, wo
            xts = {"q": [], "k": []}
            for k in range(NT):
                xqk = pX.tile([P, S], FPR, tag=f"x{k}", bufs=1, name=f"xq{k}")
                nc.sync.dma_start(xqk[:], xqT[k * P : (k + 1) * P, :])
                xts["q"].append(xqk)
            mask_sb = pE.tile([P, ST * S], BF, tag="mask")  # (~mask)^T as 0/1
            for t2 in range(ST):
                nc.sync.dma_start(
                    mask_sb[:, t2 * S : (t2 + 1) * S], maskT[t2 * P : (t2 + 1) * P, :]
                )
            wo_sb = pE.tile([P, NT * D], BF, tag="wo")
            for k in range(NT):
                nc.sync.dma_start(
                    wo_sb[:, k * D : (k + 1) * D], woT[k * P : (k + 1) * P, :]
                )
            vtags = ("cps0", "cps1", "sps", "sps")
            for c in range(2):
                vpairs = [
                    ps.tile(
                        [P, S], FP, tag=vtags[i], bufs=(2 if vtags[i] == "sps" else 1),
                        name=f"vpair{c}_{i}",
                    )
                    for i in range(4)
                ]
                vps = [
                    vpairs[st // 2][:, (st % 2) * HF : (st % 2 + 1) * HF]
                    for st in range(ST)
                ]
                for k in range(NT):
                    for st in range(ST):
                        nc.tensor.matmul(
                            vps[st],
                            xvs[k][:, st * P : (st + 1) * P],
                            wvs[k][:, c * HF : (c + 1) * HF],
                            start=(k == 0),
                            stop=False,
                        )
                for st in range(ST):
                    nc.tensor.matmul(
                        vps[st],
                        ones_row,
                        bvrow_sb[:, c * HF : (c + 1) * HF],
                        start=False,
                        stop=True,
                    )
                for st in range(ST):
                    nc.vector.tensor_copy(
                        vaug_v[:, st, c * 8 : (c + 1) * 8, 0:Dh],
                        vps[st].rearrange("p (h c2) -> p h c2", h=8),
                    )

        # ======== phase QK: Q^T / K^T projections + rope ========
        with tc.tile_pool(name="poolQK", bufs=1) as pQ:
            for name, xT, wT, bcols, is_q in (
                ("q", xqT, wqT, bqcols_sb, True),
                ("k", xkT, wkT, bkcols_sb, False),
            ):
                dst = qb_sb if is_q else kb_sb
                if not is_q:
                    # reuse the per-k x tiles; WAR lets each load start as soon
                    # as the q projection is done reading that k block
                    for k in range(NT):
                        xkk = pX.tile([P, S], FPR, tag=f"x{k}", bufs=1, name=f"xk{k}")
                        nc.sync.dma_start(xkk[:], xT[k * P : (k + 1) * P, :])
                        xts["k"].append(xkk)
                xt = xts[name]
                for j in range(NT):
                    # one 512KB DMA per j: w blocks (k=0..7) for this column
                    wj = pQ.tile([P, S], FPR, tag="wj", bufs=3)
                    nc.sync.dma_start(
                        wj[:].rearrange("p (k c) -> p k c", k=NT),
                        wT[:, j * P : (j + 1) * P].rearrange("(k p) c -> p k c", p=P),
                    )
                    rawpair = ps.tile(
                        [P, S], FP, tag="sps", bufs=2, name=f"raw{name}_{j}"
                    )
                    raws = [rawpair[:, c2 * HF : (c2 + 1) * HF] for c2 in range(2)]
                    for k in range(NT):
                        for c in range(2):
                            nc.tensor.matmul(
                                raws[c],
                                wj[:, k * P : (k + 1) * P],
                                xt[k][:, c * HF : (c + 1) * HF],
                                start=(k == 0),
                                stop=(k == NT - 1),
                            )
                    # perm pair-tile per j, tag by j parity: consecutive js
                    # never WAR-couple through the same pswap PSUM buffer
                    qpt = ps.tile(
                        [P, S], FP, tag=f"cps{j % 2}", bufs=1, name=f"perm{name}_{j}"
                    )
                    for c in range(2):
                        # evacuate with per-partition bias (pre-rope) on ACT
                        q_raw = pQ.tile([P, HF], FPR, tag="qraw", bufs=3)
                        nc.scalar.activation(
                            q_raw[:], raws[c], AF.Identity, bias=bcols[:, j : j + 1]
                        )
                        # partner-swap via permutation matmul
                        qp = qpt[:, c * HF : (c + 1) * HF]
                        nc.tensor.matmul(
                            qp, pswap_sb, q_raw[:], start=True, stop=True
                        )
                        chalf = slice(c * HF, (c + 1) * HF)
                        # k-path final add goes to idle GpSimd (its consumer
                        # is far-future attention): trims the DVE backlog that
                        # stalls the pswap matmuls through the perm-tile WAR
                        t1 = pQ.tile([P, HF], FP, tag="t1", bufs=3)
                        nc.vector.tensor_tensor(
                            t1[:], q_raw[:], cos_sb[:, chalf], op=ALU.mult
                        )
                        t2 = pQ.tile([P, HF], FP, tag="t2", bufs=3)
                        nc.vector.tensor_tensor(
                            t2[:], qp, sin_sb[:, chalf], op=ALU.mult
                        )
                        dslice = dst[:, j * S + c * HF : j * S + (c + 1) * HF]
                        if is_q:
                            nc.vector.scalar_tensor_tensor(
                                dslice,
                                t1[:],
                                ucols_sb[:, j : j + 1],
                                t2[:],
                                op0=ALU.add,
                                op1=ALU.add,
                            )
                        else:
                            nc.gpsimd.tensor_tensor(dslice, t1[:], t2[:], op=ALU.add)

        # ======== phase ATTN (software-pipelined: ctx trails scores by 1) ====
        with tc.tile_pool(name="poolAt", bufs=1) as pA:
            # rec staging: row 64 = 1/den, rows 0:64 hold the broadcast
            rb = pA.tile([P, S], FP, tag="rb")

            def emit_scores(j, tt, hi):
                base = hi * Dh
                sps = ps.tile([P, S], FP, tag="sps", bufs=2, name=f"sps{j}_{tt}_{hi}")
                for c in range(2):
                    nc.tensor.matmul(
                        sps[:, c * HF : (c + 1) * HF],
                        kb_sb[
                            base : base + Dh, j * S + tt * P : j * S + (tt + 1) * P
                        ],
                        qb_sb[
                            base : base + Dh, j * S + c * HF : j * S + (c + 1) * HF
                        ],
                        start=True,
                        stop=True,
                    )
                et = pA.tile([P, S], BF, tag="et", bufs=2, name=f"et{j}_{tt}_{hi}")
                nc.scalar.activation(et[:], sps[:], AF.Exp, scale=0.125)
                em = pA.tile([P, S], BF, tag="em", bufs=12, name=f"em{j}_{tt}_{hi}")
                nc.vector.tensor_tensor(
                    em[:], et[:], mask_sb[:, tt * S : (tt + 1) * S], op=ALU.mult
                )
                return em

            def emit_ctx(cps, j, tt, hi, em):
                h = 2 * j + hi
                for c in range(2):
                    nc.tensor.matmul(
                        cps[hi][0 : Dh + 1, c * HF : (c + 1) * HF],
                        vaug_v[:, tt, h, :],
                        em[:, c * HF : (c + 1) * HF],
                        start=(tt == 0),
                        stop=(tt == ST - 1),
                    )

            def emit_epilogue(cps, j, hi):
                # reciprocal of den (PSUM row 64), broadcast via DRAM, normalize
                h = 2 * j + hi
                nc.vector.reciprocal(rb[Dh : Dh + 1, :], cps[hi][Dh : Dh + 1, :])
                nc.sync.dma_start(rec_d[h : h + 1, :], rb[Dh : Dh + 1, :])
                nc.sync.dma_start(
                    rb[0:Dh, :], rec_d[h : h + 1, :].to_broadcast([Dh, S])
                )
                cdst = ctxu[hi * Dh : (hi + 1) * Dh, j * S : (j + 1) * S]
                nc.vector.tensor_tensor(
                    cdst, cps[hi][0:Dh, :], rb[0:Dh, :], op=ALU.mult
                )

            # one flat software pipeline across all (j, tt) units: ctx
            # trails scores by TRAIL units ACROSS j boundaries, so the ctx
            # drains interleave with the next head-pair's scores instead of
            # bunching at the j tail and starving the scalar engine.
            TRAIL = 5
            units = [(j, tt) for j in range(NT) for tt in range(ST)]
            ems = {}
            cps_by_j = {}

            def drain_unit(idx):
                j2, tt2 = units[idx]
                if tt2 == 0:
                    cps_by_j[j2] = [
                        ps.tile(
                            [P, S], FP, tag=f"cps{hi}", bufs=1, name=f"cps{j2}_{hi}"
                        )
                        for hi in range(2)
                    ]
                cps2 = cps_by_j[j2]
                if tt2 == ST - 1:
                    emit_ctx(cps2, j2, tt2, 0, ems.pop((j2, tt2, 0)))
                    emit_epilogue(cps2, j2, 0)
                    emit_ctx(cps2, j2, tt2, 1, ems.pop((j2, tt2, 1)))
                    emit_epilogue(cps2, j2, 1)
                    del cps_by_j[j2]
                else:
                    for hi in range(2):
                        emit_ctx(cps2, j2, tt2, hi, ems.pop((j2, tt2, hi)))

            for idx, (j, tt) in enumerate(units):
                for hi in range(2):
                    ems[(j, tt, hi)] = emit_scores(j, tt, hi)
                if idx >= TRAIL:
                    drain_unit(idx - TRAIL)
            for idx in range(len(units) - TRAIL, len(units)):
                drain_unit(idx)

        # ======== phase OUT: output projection ========
        with tc.tile_pool(name="poolO", bufs=1) as pO:
            ots = [
                pO.tile([P, S], FP, tag=f"ot{st}", bufs=1, name=f"ot{st}")
                for st in range(ST)
            ]
            # st pairs on the sps-tag buffers (score banks, freed right after
            # the last exp) run first so out-proj overlaps the final softmax
            # epilogue; cps-tag pairs wait for the last normalize naturally.
            ogroups = (
                ((4, 5), "sps"),
                ((6, 7), "sps"),
                ((0, 1), "cps0"),
                ((2, 3), "cps1"),
            )
            for pair, ptag in ogroups:
                for c in range(2):
                    opt = ps.tile(
                        [P, S], FP, tag=ptag, bufs=(2 if ptag == "sps" else 1),
                        name=f"opair{c}_{pair[0]}",
                    )
                    ops = {
                        st: opt[:, i * HF : (i + 1) * HF]
                        for i, st in enumerate(pair)
                    }
                    for k in range(NT):
                        for st in pair:
                            nc.tensor.matmul(
                                ops[st],
                                ctxu[:, k * S + st * P : k * S + (st + 1) * P],
                                wo_sb[:, k * D + c * HF : k * D + (c + 1) * HF],
                                start=(k == 0),
                                stop=False,
                            )
                    for st in pair:
                        nc.tensor.matmul(
                            ops[st],
                            ones_row,
                            borow_sb[:, c * HF : (c + 1) * HF],
                            start=False,
                            stop=True,
                        )
                    for st in pair:
                        nc.vector.tensor_copy(
                            ots[st][:, c * HF : (c + 1) * HF], ops[st]
                        )
                    for st in pair:
                        nc.sync.dma_start(
                            out[st * P : (st + 1) * P, c * HF : (c + 1) * HF],
                            ots[st][:, c * HF : (c + 1) * HF],
                        )

    nc.compile()
    return nc


def _host_consts():
    inv_freq = 1.0 / (ROPE_BASE ** (np.arange(0, Dh, 2, dtype=np.float64) / Dh))
    freqs = np.arange(S, dtype=np.float64)[:, None] * inv_freq[None, :]  # [S, 32]
    cos_rep = np.repeat(np.cos(freqs), 2, axis=-1)  # [S, 64]
    sin_rep = np.repeat(np.sin(freqs), 2, axis=-1)
    costab = np.empty((P, S), np.float32)
    sintab = np.empty((P, S), np.float32)
    for p in range(P):
        dl = p % Dh
        costab[p, :] = cos_rep[:, dl]
        sgn = -1.0 if (p % 2 == 0) else 1.0
        sintab[p, :] = sgn * sin_rep[:, dl]
    pswap = np.zeros((P, P), np.float32)
    for k in range(P):
        pswap[k, k ^ 1] = 1.0
    return costab, sintab, pswap


def host_in_maps(query, key, value, mask, Wq, bq, Wk, bk, Wv, bv, u_bias, Wo, bo):
    costab, sintab, pswap = _host_consts()
    u = np.asarray(u_bias, np.float32)
    smalls = np.zeros((P, 24), np.float32)
    for j in range(NT):
        smalls[:, j] = np.concatenate([u[2 * j], u[2 * j + 1]])
    smalls[:, 8:16] = np.asarray(bq, np.float32).reshape(NT, P).T
    smalls[:, 16:24] = np.asarray(bk, np.float32).reshape(NT, P).T
    rows_bf = (
        np.concatenate(
            [
                np.asarray(bv, np.float32),
                np.asarray(bo, np.float32),
                np.ones(P, np.float32),
            ]
        )
        .reshape(1, 2 * D + P)
        .astype(ml_dtypes.bfloat16)
    )
    shared = dict(
        wqT=np.ascontiguousarray(np.asarray(Wq, np.float32).T),
        wkT=np.ascontiguousarray(np.asarray(Wk, np.float32).T),
        wvT=np.ascontiguousarray(np.asarray(Wv, np.float32).T.astype(ml_dtypes.bfloat16)),
        woT=np.ascontiguousarray(np.asarray(Wo, np.float32).T.astype(ml_dtypes.bfloat16)),
        costab=costab,
        sintab=sintab,
        smalls=smalls,
        pswap=pswap,
        rows_bf=rows_bf,
    )
    in_maps = []
    for b in range(N_CORES):
        m = dict(shared)
        m["xqT"] = np.ascontiguousarray(np.asarray(query[b], np.float32).T)
        m["xkT"] = np.ascontiguousarray(np.asarray(key[b], np.float32).T)
        m["xvT"] = np.ascontiguousarray(
            np.asarray(value[b], np.float32).T.astype(ml_dtypes.bfloat16)
        )
        m["maskT"] = np.ascontiguousarray(
            (~np.asarray(mask[b], bool)).T.astype(ml_dtypes.bfloat16)
        )
        in_maps.append(m)
    return in_maps


_CACHED = {}


def kernel(query, key, value, mask, Wq, bq, Wk, bk, Wv, bv, u_bias, Wo, bo):
    if "nc" not in _CACHED:
        _CACHED["nc"] = build_nc()
    nc = _CACHED["nc"]
    in_maps = host_in_maps(
        query, key, value, mask, Wq, bq, Wk, bk, Wv, bv, u_bias, Wo, bo
    )
    res = run_bass_kernel_spmd(nc, in_maps, list(range(N_CORES)))
    return np.stack([res.results[b]["out"] for b in range(N_CORES)], axis=0)
